# revision 10
# baseline (speedup 1.0000x reference)
"""Trainium2 Bass kernel v4 for nn_Apply_on_single_area (segment_reduce).

Self-contained: accepts FULL inputs, shards areas across 8 NeuronCores
(pure data parallel), returns FULL [32768] f32 output.

Key ideas vs the v2 baseline (2.01 ms):
 - The 6-round per-channel smoothing chain collapses to ONE ACT Tanh.
   In centered v-units (v = 2pi*(x-0.5)) each round r(v) = v + sin(v)
   has slope 2 at 0 and r'(+-pi) = 0.  The 6-round composite R6 has
   slope 2^6 and saturates at +-pi; pi*tanh(64/pi * v) matches the
   transition core exactly (same slope) and its tail mismatch is
   quadratically squashed by the two exact post-combine rounds
   (super-attracting fixed points).  End-to-end l2 vs the jax
   reference: 3.2e-4 fp32 / 2.1e-3 bf16 (tolerance 2e-2).
 - The per-area sign sigma = (2sx-1)(2sy-1) from mask_index is folded
   into the sign of channel-0's centered mask on the host (the combine
   is odd), so the device kernel needs no per-area constants.
 - Engine balance: DVE keeps only the fused multiply-reduces (1x ops)
   and cheap 2x/4x passes; the mean-subtract passes run on GPSIMD; the
   variance squares accumulate on ACT (Square); tanh/sin/square are
   pinned to one ACT table set (silu_and_others) to avoid ~2.7us
   table reloads between batches.

Pipeline per batch of 128 areas (bf16 bodies, fp32 accums):
  ACT: H = tanh(128 * xc)                      [128, 4096]
  DVE: tau = H_ch0 * H_ch1                     [128, 2048]
       taupi = pi * tau            (4x tensor_scalar)
  ACT: S1 = sin(pi * tau)
  DVE: T1 = taupi + S1             (exact round 1)
  ACT: S2 = sin(T1)
  DVE: U = T1 + S2                 (exact round 2)
       M = U/(2pi) + 0.5
  stats per (mask half, axis), exact reference formulas:
    P = M * M_shift   (affine_mul_reduce -> body + msum)
    qf/qb = img_f/b * P  (affine_mul_reduce -> bodies + sums)
    rm = 1/msum (batched recip); mean = sum(q) * rm (4x ts)
    df = (q - mean) * P   [GPSIMD scalar_tensor_tensor]
    s2 = sum(df^2)        [ACT Square accumulate]
  finish phase (vectorized over batches): F = S*rm, means/vars ->
  translation variances -> loss ratio.
"""
import numpy as np
from contextlib import ExitStack

import ml_dtypes

import concourse.bass as bass
import concourse.tile as tile
from concourse import bacc, mybir
from concourse import hw_specs as _hw_specs
from concourse.bass_utils import run_bass_kernel_spmd

F32 = mybir.dt.float32
BF16 = mybir.dt.bfloat16
AF = mybir.ActivationFunctionType
OP = mybir.AluOpType

N_AREAS = 32768
DIAM = 32
PIX = DIAM * DIAM            # 1024
N_CORES = 8
APC = N_AREAS // N_CORES     # 4096 areas per core
NB = APC // 128              # 32 batches of 128 areas
TWO_PI = float(2.0 * np.pi)
PI = float(np.pi)
EPS = 1e-8
TANH_SCALE = 128.0           # = 2 * 2^6: slope of R6 in centered-x units

_CACHE = {}
TRACE = False

# ---- engine assignment knobs ----
SQ_ON_ACT = True    # variance squares via ACT Square-accum (else DVE AMR)
# df = (q - mean) * P construction:
#  "gps": DVE 4x tensor_scalar subtract + GPSIMD tensor_tensor multiply
#  "stt": single DVE scalar_tensor_tensor (1x)
DF_MODE = "stt"
DF_GPS_COUNT = 0    # how many of the 8 df passes use the gps path
ACT_MSUM = False    # msum accums via TT body + ACT Identity-accum
ACT_QB = False      # qb accums via TT body + ACT Identity-accum
GPS_TAU = False     # tau product on GPSIMD
GPS_TAUPI = True    # taupi scale on GPSIMD
GPS_U = False       # round-2 add on GPSIMD
GPS_M = True        # m affine on GPSIMD
POOLS = {"x": 3, "h": 3, "t": 9, "m": 4, "img": 4, "pq": 32, "tiny": 24}


def _pin_act_tables():
    """Keep Tanh/Sin/Square resolvable only via silu_and_others so the
    table-load chooser never alternates sets between tanh and sin calls.
    Set ids (dict order) are preserved; other sets just lose the three
    functions we use, which nothing else in this kernel calls."""
    if getattr(_hw_specs, "_ant_pinned_tables", False):
        return
    orig = _hw_specs.get_activation_tables
    pin = {AF.Tanh, AF.Sin, AF.Square}

    def patched(arch):
        t = orig(arch)
        home = t.get("silu_and_others")
        if home and pin <= home:
            t = {k: (v if k == "silu_and_others" else v - pin)
                 for k, v in t.items()}
        return t

    _hw_specs.get_activation_tables = patched
    bacc.get_activation_tables = patched
    _hw_specs._ant_pinned_tables = True


_pin_act_tables()


def _stats_axis(nc, pools, mAP, m3AP, imgT, img3, axis, S, msum_col):
    """Emit the multiply-reduce part of one (mask-half, axis) unit.
    Writes sum_qf, sum_qb into S cols [4*axis, 4*axis+1], msum into
    msum_col, and returns (P, qf, qb) tiles for the variance stage."""
    pq = pools["pq"]
    if axis == 0:
        a0, a1 = mAP[:, 64:1024], mAP[:, 0:960]
        i_f, i_b = imgT[:, 64:1024], imgT[:, 0:960]
        shp3 = None
    else:
        a0, a1 = m3AP[:, :, 2:DIAM], m3AP[:, :, 0 : DIAM - 2]
        i_f, i_b = img3[:, :, 2:DIAM], img3[:, :, 0 : DIAM - 2]
        shp3 = True

    def v3(tl):
        return tl[:] if shp3 is None else tl[:].rearrange(
            "p (r c) -> p r c", c=DIAM - 2)

    col = 4 * axis
    P = pq.tile([128, 960], BF16, tag="pq")
    qf = pq.tile([128, 960], BF16, tag="pq")
    qb = pq.tile([128, 960], BF16, tag="pq")
    if ACT_MSUM:
        nc.vector.tensor_tensor(v3(P), a0, a1, OP.mult)
        junk = pq.tile([128, 960], BF16, tag="pq")
        nc.scalar.activation(junk[:], P[:], AF.Identity,
                             accum_out=msum_col)
    else:
        nc.vector.affine_mul_reduce(out=v3(P), accum_out=msum_col, in0=a0,
                                    in1=a1, scale=1.0, bias=0.0)
    nc.vector.affine_mul_reduce(out=v3(qf), accum_out=S[:, col : col + 1],
                                in0=i_f, in1=v3(P), scale=1.0, bias=0.0)
    if ACT_QB:
        nc.vector.tensor_tensor(v3(qb), i_b, v3(P), OP.mult)
        junkb = pq.tile([128, 960], BF16, tag="pq")
        nc.scalar.activation(junkb[:], qb[:], AF.Identity,
                             accum_out=S[:, col + 1 : col + 2])
    else:
        nc.vector.affine_mul_reduce(out=v3(qb),
                                    accum_out=S[:, col + 1 : col + 2],
                                    in0=i_b, in1=v3(P), scale=1.0, bias=0.0)
    return P, qf, qb


def _variance_stage(nc, pools, units, rm4, S_of):
    """Per batch: means from the accumulated sums, then df = (q-mean)*P on
    GPSIMD and Square-accumulate on ACT."""
    pq = pools["pq"]
    tiny = pools["tiny"]
    for k, (h, axis, P, qf, qb) in enumerate(units):
        S = S_of(h)
        col = 4 * axis
        rmc = rm4[:, 2 * h + axis : 2 * h + axis + 1]
        for d, q in ((0, qf), (1, qb)):
            mean = tiny.tile([128, 1], F32, tag="tiny")
            nc.vector.tensor_scalar(mean[:], S[:, col + d : col + d + 1],
                                    rmc, None, OP.mult)
            df = pq.tile([128, 960], BF16, tag="pq")
            if DF_MODE == "gps" and 2 * k + d < DF_GPS_COUNT:
                qs = pq.tile([128, 960], BF16, tag="pq")
                nc.vector.tensor_scalar(qs[:], q[:], mean[:], None,
                                        OP.subtract)
                nc.gpsimd.tensor_tensor(df[:], qs[:], P[:], OP.mult)
            else:
                nc.vector.scalar_tensor_tensor(df[:], q[:], mean[:], P[:],
                                               OP.subtract, OP.mult)
            if SQ_ON_ACT:
                junk = pq.tile([128, 960], BF16, tag="pq")
                nc.scalar.activation(junk[:], df[:], AF.Square, scale=1.0,
                                     accum_out=S[:, col + 2 + d : col + 3 + d])
            else:
                nc.vector.affine_mul_reduce(
                    out=df[:], accum_out=S[:, col + 2 + d : col + 3 + d],
                    in0=df[:], in1=df[:], scale=1.0, bias=0.0)


def build_body4(nc, drams, nbatches, full):
    xA_d, xB_d, img_d, out_d = drams
    with tile.TileContext(nc) as tc, ExitStack() as ctx:
        pools = {
            k: ctx.enter_context(tc.tile_pool(name=k, bufs=POOLS[k]))
            for k in ("x", "h", "t", "m", "img", "pq", "tiny")
        }
        pools.update({
            "acc": ctx.enter_context(tc.tile_pool(name="acc", bufs=1)),
            "fin": ctx.enter_context(tc.tile_pool(name="fin", bufs=1)),
        })

        nrot = min(nbatches, 32)
        Sall = pools["acc"].tile([128, nrot * 16], F32, tag="sall")
        Rall = pools["acc"].tile([128, nrot * 4], F32, tag="rall")

        for b in range(nbatches):
            r0 = b * 128 if full else 0
            X = pools["x"].tile([128, 4096], BF16, tag="x")
            nc.sync.dma_start(X[:, 0:2048], xA_d.ap()[r0 : r0 + 128, :])
            nc.sync.dma_start(X[:, 2048:4096], xB_d.ap()[r0 : r0 + 128, :])
            img = pools["img"].tile([128, PIX], BF16, tag="img")
            nc.sync.dma_start(img[:], img_d.ap()[r0 : r0 + 128, :])

            # chain: H = tanh(128*xc); two exact rounds on pi*(H0*H1); M
            H = pools["h"].tile([128, 4096], BF16, tag="h")
            nc.scalar.activation(H[:], X[:], AF.Tanh, scale=TANH_SCALE)
            Hv = H[:].rearrange("p (m c two) -> p m c two", m=2, two=2)
            tau = pools["t"].tile([128, 2048], BF16, tag="t")
            tauv = tau[:].rearrange("p (m c) -> p m c", m=2)
            teng = nc.gpsimd if GPS_TAU else nc.vector
            teng.tensor_tensor(tauv, Hv[:, :, :, 0], Hv[:, :, :, 1], OP.mult)
            taupi = pools["t"].tile([128, 2048], BF16, tag="t")
            tpeng = nc.gpsimd if GPS_TAUPI else nc.vector
            tpeng.tensor_scalar(taupi[:], tau[:], PI, None, OP.mult)
            S1 = pools["t"].tile([128, 2048], BF16, tag="t")
            nc.scalar.activation(S1[:], tau[:], AF.Sin, scale=PI)
            T1 = pools["t"].tile([128, 2048], BF16, tag="t")
            nc.vector.tensor_tensor(T1[:], taupi[:], S1[:], OP.add)
            S2 = pools["t"].tile([128, 2048], BF16, tag="t")
            nc.scalar.activation(S2[:], T1[:], AF.Sin, scale=1.0)
            U = pools["t"].tile([128, 2048], BF16, tag="t")
            ueng = nc.gpsimd if GPS_U else nc.vector
            ueng.tensor_tensor(U[:], T1[:], S2[:], OP.add)
            M = pools["m"].tile([128, 2048], BF16, tag="m")
            meng = nc.gpsimd if GPS_M else nc.vector
            meng.tensor_scalar(M[:], U[:], 1.0 / TWO_PI, 0.5, OP.mult,
                               OP.add)

            img3 = img[:].rearrange("p (r c) -> p r c", c=DIAM)
            br = b % nrot

            def S_of(h):
                return Sall[:, (br * 2 + h) * 8 : (br * 2 + h) * 8 + 8]

            msum4 = pools["tiny"].tile([128, 4], F32, tag="tiny4")
            units = []
            for h in range(2):
                mh = M[:, h * PIX : (h + 1) * PIX]
                m3 = mh.rearrange("p (r c) -> p r c", c=DIAM)
                for axis in (0, 1):
                    P, qf, qb = _stats_axis(
                        nc, pools, mh, m3, img[:], img3, axis, S_of(h),
                        msum4[:, 2 * h + axis : 2 * h + axis + 1])
                    units.append((h, axis, P, qf, qb))
            rm4 = Rall[:, br * 4 : br * 4 + 4]
            nc.vector.reciprocal(rm4, msum4[:])
            _variance_stage(nc, pools, units, rm4, S_of)

        # ---- finish phase (vectorized over batches) ----
        fin = pools["fin"]
        nb = nrot
        # F = S * rm  (rm broadcast over the 4 cols of each unit)
        Fall = fin.tile([128, nb * 16], F32, tag="fall")
        Sv = Sall[:].rearrange("p (u f) -> p u f", f=4)     # u = (br, h, ax)
        Fv = Fall[:].rearrange("p (u f) -> p u f", f=4)
        Rv = Rall[:].rearrange("p (u one) -> p u one", one=1).broadcast_to(
            (128, nb * 4, 4))
        nc.vector.tensor_tensor(Fv, Sv, Rv, OP.mult)
        FS = fin.tile([128, nb * 2 * 2], F32, tag="FS")
        Fsum_view = Fall[:].rearrange("p (u ax mv f) -> p u mv ax f",
                                      ax=2, mv=2, f=2)
        FSv = FS[:].rearrange("p (u mv) -> p u mv", mv=2)
        nc.vector.tensor_reduce(FSv, Fsum_view, mybir.AxisListType.XY, OP.add)
        F2 = fin.tile([128, nb * 16], F32, tag="F2")
        nc.vector.tensor_tensor(F2[:], Fall[:], Fall[:], OP.mult)
        F2S = fin.tile([128, nb * 2 * 2], F32, tag="F2S")
        F2sum_view = F2[:].rearrange("p (u ax mv f) -> p u mv ax f",
                                     ax=2, mv=2, f=2)
        F2Sv = F2S[:].rearrange("p (u mv) -> p u mv", mv=2)
        nc.vector.tensor_reduce(F2Sv, F2sum_view, mybir.AxisListType.XY,
                                OP.add)
        SS = fin.tile([128, nb * 2 * 2], F32, tag="SS")
        nc.vector.tensor_tensor(SS[:], FS[:], FS[:], OP.mult)
        Lmv = fin.tile([128, nb * 2 * 2], F32, tag="Lmv")
        nc.vector.scalar_tensor_tensor(Lmv[:], F2S[:], 4.0, SS[:],
                                       OP.mult, OP.subtract)
        L = fin.tile([128, nb * 2], F32, tag="L")
        Lv = L[:].rearrange("p (u one) -> p u one", one=1)
        nc.vector.tensor_reduce(Lv, Lmv[:].rearrange(
            "p (u mv) -> p u mv", mv=2), mybir.AxisListType.X, OP.add)
        Lm = L[:].rearrange("p (b two) -> p b two", two=2)
        den = fin.tile([128, nb], F32, tag="den")
        nc.vector.tensor_tensor(den[:], Lm[:, :, 0], Lm[:, :, 1], OP.add)
        den2 = fin.tile([128, nb], F32, tag="den2")
        nc.vector.tensor_scalar(den2[:], den[:], 32.0 * EPS, None, OP.add)
        rden = fin.tile([128, nb], F32, tag="rden")
        nc.vector.reciprocal(rden[:], den2[:])
        outc = fin.tile([128, nb], F32, tag="outc")
        nc.vector.tensor_tensor(outc[:], Lm[:, :, 0], rden[:], OP.mult)
        nc.sync.dma_start(out_d.ap()[:, 0:nb], outc[:])


def build_nc2(nbatches=NB, full=True):
    nc = bacc.Bacc("TRN2", target_bir_lowering=False, debug=False,
                   num_devices=N_CORES)
    rows = nbatches * 128 if full else 128
    xA_d = nc.dram_tensor("xA", [rows, 2 * PIX], BF16, kind="ExternalInput")
    xB_d = nc.dram_tensor("xB", [rows, 2 * PIX], BF16, kind="ExternalInput")
    img_d = nc.dram_tensor("img", [rows, PIX], BF16, kind="ExternalInput")
    out_d = nc.dram_tensor("out", [128, nbatches], F32,
                           kind="ExternalOutput")
    build_body4(nc, (xA_d, xB_d, img_d, out_d), nbatches, full)
    nc.finalize()
    return nc


def _prep(resized_image, mask_combined, mask_combined_alt, mask_index):
    bf = ml_dtypes.bfloat16
    idx = np.asarray(mask_index).astype(np.int64)
    sig = (1.0 - 2.0 * ((idx % 2) ^ (idx // 2))).astype(np.float32)

    def center(m):
        x = np.asarray(m, np.float32).reshape(N_AREAS, PIX, 2) - np.float32(
            0.5)
        x[:, :, 0] *= sig[:, None]
        return x.reshape(N_AREAS, 2 * PIX).astype(bf)

    xA = center(mask_combined)
    xB = center(mask_combined_alt)
    img = np.asarray(resized_image, np.float32).reshape(
        N_AREAS, PIX).astype(bf)
    return xA, xB, img


def kernel(resized_image, mask_combined, mask_combined_alt, mask_index):
    xA, xB, img = _prep(resized_image, mask_combined,
                        mask_combined_alt, mask_index)
    if "nc" not in _CACHE:
        _CACHE["nc"] = build_nc2()
    nc = _CACHE["nc"]
    in_maps = []
    for c in range(N_CORES):
        s = slice(c * APC, (c + 1) * APC)
        in_maps.append({"xA": xA[s], "xB": xB[s], "img": img[s]})
    res = run_bass_kernel_spmd(nc, in_maps, core_ids=list(range(N_CORES)),
                               trace=TRACE)
    outs = []
    for c in range(N_CORES):
        o = res.results[c]["out"]          # [128, NB]
        outs.append(np.ascontiguousarray(o.T).reshape(APC))
    return np.concatenate(outs).astype(np.float32)
